# revision 38
# baseline (speedup 1.0000x reference)
"""Trainium2 Bass kernel: RoPE causal attention (B=1,S=2048,D=4096,H=32).

Tensor-parallel over heads on 8 NeuronCores: core c owns heads [4c,4c+4).
Fully fused single-pass kernel, no DRAM round trips between stages:

- Host passes x pre-transposed (xT [D,S]) and weights pre-transposed in
  bf16, with wq/wk rows pair-permuted per head (evens then odds) so RoPE
  pairs land in contiguous partition halves. cos/sin are precomputed on
  host, transposed and duplicated into both partition halves [128, S].
- Projections compute qT/kT in [hd, s] layout directly (lhsT = w tiles,
  rhs = xT tiles) and v in [s, hd] layout; RoPE is fused into the
  PSUM->SBUF evacuation: one ACT copy releases the PSUM bank, a DVE
  partition-swapped copy pairs the rotation halves, then one DVE mult,
  one DVE scalar_tensor_tensor and one GpSimd add per head.
- Attention computes scores transposed [sk, sq] so that exp(scores)
  (written by ACT straight into SBUF) is directly the lhsT of the P@V
  matmul -- no PE transposes, no probability copies. Softmax skips the
  max subtraction (|scores/sqrt(hd)| <= ~10, exp cannot overflow) and
  folds normalization in after P@V: a ones-matmul gives the row-sum
  broadcast across partitions, one reciprocal + one multiply normalize.
- wo consumes the attention output from SBUF, accumulating over the 4
  local head slices; partial y [S, D] goes out in bf16 and the host sums
  the 8 per-core partials (float32 accumulation).
- q/k weight streams load 4 quads per DMA at c>0 (512KB contiguous) to
  relieve the sync-queue descriptor-issue engine (SP), which was the #2
  busiest engine.
- The last chunk's attention phase is ACT(exp)-bound, so it is fully
  software-pipelined: head 0's scores+exp run before the v projection,
  head 1's interleave into v-proj's second x-half (PT buffer on the
  then-dead xcA slot), and heads 2/3's scores pair tile-by-tile with
  heads 0/1's PV matmuls (PT buffers on the dead xcB and recycled pt
  slots). Row-sums hoist into the next score stream so the sum-chain
  slots recycle without queue-order deadlocks.
- Chunk-0 x rides the gpsimd DMA queue (the sync queue's issue engine
  saturates on weights at startup); wo accumulators rotate through 4
  PSUM banks (pyo x2 + idle pat/prs) so the first output tiles don't
  wait for the attention tail.
  Single-pass v projection (4 PSUM banks) at c<3 halves wv traffic.
  Sim (CoreSim no_exec): 504.7us vs 520.8us staged baseline; PE busy
  98.7% against a 498.7us bf16 PE cycle floor.
"""

import math
from contextlib import ExitStack
import numpy as np
import ml_dtypes

import concourse.bass as bass
import concourse.mybir as mybir
import concourse.tile as tile
from concourse import bacc
from concourse.bass_utils import run_bass_kernel_spmd

B, S, D, H, HD = 1, 2048, 4096, 32, 128
NCORES = 8
HL = H // NCORES          # 4 heads per core
DL = HL * HD              # 512 local head dims
NT = S // 128             # 16 seq tiles of 128
NCH = S // 512            # 4 seq chunks of 512
KD = D // 128             # 32 contraction tiles
SCALE = 1.0 / math.sqrt(HD)
F32 = mybir.dt.float32
BF16 = mybir.dt.bfloat16
MUL = mybir.AluOpType.mult
ADD = mybir.AluOpType.add
SUB = mybir.AluOpType.subtract
EXP = mybir.ActivationFunctionType.Exp

BFNP = ml_dtypes.bfloat16

_CACHE = {}


def _build():
    nc = bacc.Bacc(None, target_bir_lowering=False, debug=False)
    xT_t = nc.dram_tensor("xT", [D, S], BF16, kind="ExternalInput")
    cos_t = nc.dram_tensor("cosT", [128, S], F32, kind="ExternalInput")
    sin_t = nc.dram_tensor("sinT", [128, S], F32, kind="ExternalInput")
    wq_t = nc.dram_tensor("wq", [D, DL], BF16, kind="ExternalInput")
    wk_t = nc.dram_tensor("wk", [D, DL], BF16, kind="ExternalInput")
    wv_t = nc.dram_tensor("wv", [D, DL], BF16, kind="ExternalInput")
    wo_t = nc.dram_tensor("wo", [DL, D], BF16, kind="ExternalInput")
    mk_t = nc.dram_tensor("maskT", [128, 128], BF16, kind="ExternalInput")
    y_t = nc.dram_tensor("y", [S, D], BF16, kind="ExternalOutput")

    xT_r = xT_t.ap().rearrange("(a r) s -> r a s", r=128)   # [128, 32, 2048]
    # wq/wk are host-packed as [head, quad, 128, 512] flat blocks so each
    # quad DMA is one [128, 512] tile with 1KB contiguous runs
    wq_r = wq_t.ap().rearrange("(m q r) n -> m q r n", m=HL, q=KD // 4)
    wk_r = wk_t.ap().rearrange("(m q r) n -> m q r n", m=HL, q=KD // 4)
    # batched view: 4 quads per DMA ([128, 4, 512], 512KB contiguous in DRAM)
    # to cut sync-queue descriptor-issue load (SP was 64 issues/chunk)
    wq_b = wq_t.ap().rearrange("(m q r) n -> m r q n", m=HL, q=KD // 4)
    wk_b = wk_t.ap().rearrange("(m q r) n -> m r q n", m=HL, q=KD // 4)
    wv_r = wv_t.ap().rearrange("(a r) n -> r a n", r=128)

    with tile.TileContext(nc) as tc, ExitStack() as es:
        pers = es.enter_context(tc.tile_pool(name="pers", bufs=1))
        xp = es.enter_context(tc.tile_pool(name="xp", bufs=1))
        wp = es.enter_context(tc.tile_pool(name="wp", bufs=8))
        wpv = es.enter_context(tc.tile_pool(name="wpv", bufs=4))
        rt = es.enter_context(tc.tile_pool(name="rt", bufs=2))
        rtc = es.enter_context(tc.tile_pool(name="rtc", bufs=1))
        ptp = es.enter_context(tc.tile_pool(name="ptp", bufs=1))
        atp = es.enter_context(tc.tile_pool(name="atp", bufs=5))
        rip = es.enter_context(tc.tile_pool(name="rip", bufs=2))
        smp = es.enter_context(tc.tile_pool(name="smp", bufs=1))
        yp = es.enter_context(tc.tile_pool(name="yp", bufs=2))
        ppj = es.enter_context(tc.tile_pool(name="ppj", bufs=2, space="PSUM"))
        psc = es.enter_context(tc.tile_pool(name="psc", bufs=2, space="PSUM"))
        prs = es.enter_context(tc.tile_pool(name="prs", bufs=1, space="PSUM"))
        pat = es.enter_context(tc.tile_pool(name="pat", bufs=1, space="PSUM"))
        pyo = es.enter_context(tc.tile_pool(name="pyo", bufs=2, space="PSUM"))
        if True:
            qT = pers.tile([128, HL, S], BF16)      # [hd, h, s]
            kT = pers.tile([128, HL, S], BF16)
            vS = pers.tile([128, NT, DL], BF16)     # [s%128, s//128, dl]
            woT = pers.tile([128, HL, D], BF16)     # [dl%128, dl//128, o]
            cos2 = pers.tile([128, S], F32)   # cos duplicated in both halves
            sin2 = pers.tile([128, S], F32)
            maskT = pers.tile([128, 128], BF16)
            ones = pers.tile([128, 128], BF16)
            sign = pers.tile([128, 1], F32)   # -1 in rows 0:64, +1 in rows 64:128

            # startup: chunk-0 x rides the gpsimd queue (weights saturate the
            # sync queue's issue engine); cos/sin/mask/wo follow behind it,
            # still well ahead of their first consumers (rope ~30us, wo ~100us)
            nc.vector.memset(ones[:], 1.0)
            nc.vector.memset(sign[0:64, :], -1.0)
            nc.vector.memset(sign[64:128, :], 1.0)

            for c in range(NCH):
                cs = slice(c * 512, (c + 1) * 512)
                g = c

                # ---- x chunk load (xT is read once total), interleaved with
                # the first q-head's weight quads so PE can start immediately
                # x chunk in two k-halves so each half releases (and the next
                # chunk's load starts) as soon as its last reader finishes
                xcA = xp.tile([128, KD // 2, 512], BF16, tag="xca")
                xcB = xp.tile([128, KD // 2, 512], BF16, tag="xcb")

                def xck(k):
                    return (xcA, k) if k < KD // 2 else (xcB, k - KD // 2)

                wts0 = []
                wtsk = []
                wtsq1 = []
                wtsk1 = []
                wtsq2 = []
                for q4 in range(KD // 4):
                    if c == 0:
                        wt = wp.tile([128, 512], BF16, tag="w")
                        nc.sync.dma_start(out=wt[:], in_=wq_r[0, q4])
                        wts0.append(wt)
                    elif q4 % 4 == 0:
                        wt = wp.tile([128, 4, 512], BF16, tag="wb", bufs=2, name="wbt")
                        nc.sync.dma_start(out=wt[:], in_=wq_b[0, :, q4:q4 + 4, :])
                        wts0.append(wt)
                    xh, kq = (xcA, q4) if q4 < 4 else (xcB, q4 - 4)
                    xeng = nc.gpsimd if c == 0 else nc.sync
                    if q4 == 0:
                        # split the first quad so the first matmul starts sooner
                        xeng.dma_start(out=xh[:, 0, :], in_=xT_r[:, 0, cs])
                        xeng.dma_start(out=xh[:, 1:4, :], in_=xT_r[:, 1:4, cs])
                    else:
                        xeng.dma_start(
                            out=xh[:, kq * 4:(kq + 1) * 4, :],
                            in_=xT_r[:, q4 * 4:(q4 + 1) * 4, cs],
                        )
                    if c == 0:
                        # chunk 0: no prior attention exists, so stream three
                        # more weight streams (k0, q1, k1) and use the idle
                        # attention PSUM banks -- four PE consumers fully
                        # hide the x-chunk DMA
                        for w_r2, m2, lst in ((wk_r, 0, wtsk), (wq_r, 1, wtsq1),
                                              (wk_r, 1, wtsk1)):
                            wt2 = wp.tile([128, 512], BF16, tag="w", name="wt2")
                            nc.sync.dma_start(out=wt2[:], in_=w_r2[m2, q4])
                            lst.append(wt2)

                if c == 0:
                    # behind the x chunk on the gpsimd queue: rope tables
                    # (first use ~30us), mask, and wo (first use ~100us)
                    nc.gpsimd.dma_start(out=cos2[:], in_=cos_t[:, :])
                    nc.gpsimd.dma_start(out=sin2[:], in_=sin_t[:, :])
                    nc.gpsimd.dma_start(out=maskT[:], in_=mk_t[:, :])
                    for hw_ in range(HL):
                        nc.gpsimd.dma_start(out=woT[:, hw_, :],
                                            in_=wo_t[hw_ * 128:(hw_ + 1) * 128, :])

                # ---- q/k projections + fused rope ----
                # rope evac: one ACT copy pc releases the PSUM bank;
                # pcs = pair-swapped copy (DVE partition-shift), then
                # u = pc*cos, w2 = (pcs*sign)*sin, dst = u + w2 (Pool).
                def rope_evac(ps, dstT, m):
                    pc = rtc.tile([128, 512], F32, tag="pc", name="pc")
                    pcs = rtc.tile([128, 512], F32, tag="pcs", name="pcs")
                    nc.scalar.copy(out=pc[:], in_=ps[:])
                    nc.vector.tensor_copy(out=pcs[0:64, :], in_=pc[64:128, :])
                    nc.vector.tensor_copy(out=pcs[64:128, :], in_=pc[0:64, :])
                    u = rt.tile([128, 512], F32, tag="ra", name="u")
                    w2 = rt.tile([128, 512], F32, tag="rb", name="w2")
                    nc.vector.tensor_tensor(out=u[:], in0=pc[:], in1=cos2[:, cs], op=MUL)
                    nc.vector.scalar_tensor_tensor(out=w2[:], in0=pcs[:], scalar=sign[:],
                                                   in1=sin2[:, cs], op0=MUL, op1=MUL)
                    nc.gpsimd.tensor_tensor(out=dstT[:, m, cs], in0=u[:], in1=w2[:], op=ADD)

                if c == 0:
                    # quad stream-following pass: q0/k0 in the projection
                    # banks, q1/k1 in the (idle) attention banks
                    psq0 = ppj.tile([128, 512], F32, tag="pj", name="psq0")
                    psk0 = ppj.tile([128, 512], F32, tag="pj", name="psk0")
                    psq1 = pat.tile([128, 512], F32, tag="at", name="psq1")
                    psk1 = prs.tile([128, 512], F32, tag="rs", name="psk1")
                    for q4 in range(KD // 4):
                        for kk in range(4):
                            k = q4 * 4 + kk
                            xh, kh = xck(k)
                            for wt, psx in ((wts0[q4], psq0), (wtsk[q4], psk0),
                                            (wtsq1[q4], psq1), (wtsk1[q4], psk1)):
                                nc.tensor.matmul(
                                    psx[:],
                                    wt[:, kk * 128:(kk + 1) * 128],
                                    xh[:, kh, :],
                                    start=(k == 0), stop=(k == KD - 1),
                                )
                    rope_evac(psq0, qT, 0)
                    rope_evac(psk0, kT, 0)
                    rope_evac(psq1, qT, 1)
                    rope_evac(psk1, kT, 1)

                for w_b, dstT in ((wq_b, qT), (wk_b, kT)):
                    for m in range(HL):
                        if c == 0 and m < 2:
                            continue  # done in the quad pass above
                        reuse0 = (c > 0 and dstT is qT and m == 0)
                        ps = ppj.tile([128, 512], F32, tag="pj")
                        for q4 in range(KD // 4):
                            if c == 0:
                                # chunk 0: per-quad loads — the sync queue is
                                # issue-saturated at startup, and small loads
                                # keep the prefetch granularity fine
                                wq1 = wp.tile([128, 512], BF16, tag="w", name="wq1")
                                nc.sync.dma_start(
                                    out=wq1[:],
                                    in_=(wq_r if dstT is qT else wk_r)[m, q4])
                                wbv = wq1[:]
                            elif q4 % 4 == 0:
                                if reuse0:
                                    wb = wts0[q4 // 4]
                                else:
                                    wb = wp.tile([128, 4, 512], BF16, tag="wb",
                                                 bufs=2, name="wb")
                                    nc.sync.dma_start(
                                        out=wb[:], in_=w_b[m, :, q4:q4 + 4, :])
                                wbv = wb[:, q4 % 4, :]
                            else:
                                wbv = wb[:, q4 % 4, :]
                            for kk in range(4):
                                k = q4 * 4 + kk
                                xh, kh = xck(k)
                                nc.tensor.matmul(
                                    ps[:],
                                    wbv[:, kk * 128:(kk + 1) * 128],
                                    xh[:, kh, :],
                                    start=(k == 0), stop=(k == KD - 1),
                                )
                        rope_evac(ps, dstT, m)

                # ---- last chunk only: heads 0-1 scores+exp emitted around
                # the v projection, so the ACT exp backlog (the g=3
                # bottleneck) drains while PE runs the v matmuls instead of
                # stalling the attention phase. PV still runs after v-proj.
                last_g = (g == NCH - 1)
                nsk = 4 * g + 4

                def score_exp(h, t, PTb, sme_, smo_, sc):
                    j0 = t - 4 * g
                    lo = max(j0, 0) * 128
                    nc.tensor.matmul(
                        sc[:, lo:512],
                        kT[:, h, t * 128:(t + 1) * 128],
                        qT[:, h, g * 512 + lo:(g + 1) * 512],
                        start=True, stop=True,
                    )
                    nc.scalar.activation(PTb[:, t, lo:512], sc[:, lo:512],
                                         EXP, scale=SCALE)
                    if j0 >= 0:
                        nc.vector.tensor_tensor(
                            out=PTb[:, t, lo:lo + 128],
                            in0=PTb[:, t, lo:lo + 128],
                            in1=maskT[:], op=MUL,
                        )
                    eng = nc.vector if t % 2 == 0 else nc.gpsimd
                    smx = sme_ if t % 2 == 0 else smo_
                    if t < 2:
                        if lo > 0:
                            eng.memset(smx[:, 0:lo], 0.0)
                        eng.tensor_copy(out=smx[:, lo:512], in_=PTb[:, t, lo:512])
                    else:
                        eng.tensor_tensor(out=smx[:, lo:512], in0=smx[:, lo:512],
                                          in1=PTb[:, t, lo:512], op=ADD)

                def pre_sc(t):
                    if t % 4 < 2:
                        return psc.tile([128, 512], F32, tag="sc", name="psct")
                    return pyo.tile([128, 512], F32, tag="yo", name="pyot")

                pre = {}
                if last_g:
                    PT0 = ptp.tile([128, NT, 512], BF16, tag="pt", name="PT0")
                    sme0 = smp.tile([128, 512], F32, tag="sme", name="sme0")
                    smo0 = smp.tile([128, 512], F32, tag="smo", name="smo0")
                    for t in range(nsk):
                        score_exp(0, t, PT0, sme0, smo0, pre_sc(t))

                # ---- v projection. Chunks 0-2: single pass over K with all
                # 4 s-tiles in 4 PSUM banks (ppj x2 + pat/prs, idle here) --
                # halves the wv DMA traffic vs the 2-pass form. Last chunk
                # keeps the 2-pass form: head-1 scores interleave into the
                # second x-half of the jp=1 pass (their PT buffer reuses the
                # xcA slot, dead from that point on), and head-0's row-sum is
                # hoisted in to release the sum-chain slots for head 1 ----
                if not last_g:
                    psA = ppj.tile([128, 512], F32, tag="pj", name="psA1")
                    psB = ppj.tile([128, 512], F32, tag="pj", name="psB1")
                    psC = pat.tile([128, 512], F32, tag="at", name="psC1")
                    psD = prs.tile([128, 512], F32, tag="rs", name="psD1")
                    vbanks = (psA, psB, psC, psD)
                    for q4 in range(KD // 4):
                        wt = wpv.tile([128, 4, 512], BF16, tag="wv")
                        nc.sync.dma_start(out=wt[:], in_=wv_r[:, q4 * 4:(q4 + 1) * 4, :])
                        for kk in range(4):
                            k = q4 * 4 + kk
                            xh, kh = xck(k)
                            for j, psx in enumerate(vbanks):
                                nc.tensor.matmul(
                                    psx[:],
                                    xh[:, kh, j * 128:(j + 1) * 128],
                                    wt[:, kk, :],
                                    start=(k == 0), stop=(k == KD - 1),
                                )
                    for j, psx in enumerate(vbanks):
                        nc.vector.tensor_copy(out=vS[:, 4 * c + j, :], in_=psx[:])
                for jp in range(2 if last_g else 0):
                    psA = ppj.tile([128, 512], F32, tag="pj")
                    psB = ppj.tile([128, 512], F32, tag="pj")
                    for q4 in range(KD // 4):
                        wt = wpv.tile([128, 4, 512], BF16, tag="wv")
                        nc.sync.dma_start(out=wt[:], in_=wv_r[:, q4 * 4:(q4 + 1) * 4, :])
                        if last_g and jp == 1 and q4 == KD // 8:
                            stot0 = rip.tile([128, 512], BF16, tag="stot", name="stot0")
                            nc.vector.tensor_tensor(out=stot0[:], in0=sme0[:],
                                                    in1=smo0[:], op=ADD)
                            rs0 = prs.tile([128, 512], F32, tag="rs", name="rs0")
                            nc.tensor.matmul(rs0[:], ones[:], stot0[:],
                                             start=True, stop=True)
                            ri0 = rip.tile([128, 512], F32, tag="ri", name="ri0")
                            nc.vector.reciprocal(ri0[:], rs0[:])
                            PT1 = xp.tile([128, NT, 512], BF16, tag="xca", name="PT1")
                            sme1 = smp.tile([128, 512], F32, tag="sme", name="sme1")
                            smo1 = smp.tile([128, 512], F32, tag="smo", name="smo1")
                            pre[0] = (PT0, sme0, smo0, ri0)
                            pre[1] = (PT1, sme1, smo1, None)
                        for kk in range(4):
                            k = q4 * 4 + kk
                            if last_g and jp == 1 and q4 >= KD // 8:
                                t1 = (q4 - KD // 8) * 4 + kk
                                score_exp(1, t1, PT1, sme1, smo1, pre_sc(t1))
                            xh, kh = xck(k)
                            for jj, psx in ((0, psA), (1, psB)):
                                j = 2 * jp + jj
                                nc.tensor.matmul(
                                    psx[:],
                                    xh[:, kh, j * 128:(j + 1) * 128],
                                    wt[:, kk, :],
                                    start=(k == 0), stop=(k == KD - 1),
                                )
                    nc.vector.tensor_copy(out=vS[:, 4 * c + 2 * jp, :], in_=psA[:])
                    nc.vector.tensor_copy(out=vS[:, 4 * c + 2 * jp + 1, :], in_=psB[:])

                # ---- causal attention for q-block g (512 queries) ----
                def plo(t):
                    return max(t - 4 * g, 0) * 128

                def pv(at_, h, t, PTb):
                    lo = plo(t)
                    nc.tensor.matmul(
                        at_[:, lo:512], vS[:, t, h * 128:(h + 1) * 128],
                        PTb[:, t, lo:512],
                        start=(t == 0), stop=(t == nsk - 1))

                def rowsum(stot_, name):
                    rs_ = prs.tile([128, 512], F32, tag="rs", name="rs" + name)
                    nc.tensor.matmul(rs_[:], ones[:], stot_[:], start=True, stop=True)
                    ri_ = rip.tile([128, 512], F32, tag="ri", name="ri" + name)
                    nc.vector.reciprocal(ri_[:], rs_[:])
                    return ri_

                def stot_of(sme_, smo_, name):
                    stot_ = rip.tile([128, 512], BF16, tag="stot", name="stot" + name)
                    nc.vector.tensor_tensor(out=stot_[:], in0=sme_[:], in1=smo_[:], op=ADD)
                    return stot_

                def norm(at_, ri_, name):
                    an_ = atp.tile([128, 512], BF16, tag="attn", name="an" + name)
                    nc.vector.tensor_tensor(out=an_[:], in0=at_[:], in1=ri_[:], op=MUL)
                    return an_

                if last_g:
                    # software-pipelined last block: h0/h1 probabilities are
                    # ready (computed around v-proj). h2's scores pair with
                    # h0's PV matmuls and h3's with h1's, so their exps hide
                    # behind PE work; each head's row-sum hoists into the next
                    # score stream (DVE queue order: stot before the sums that
                    # reuse its input slots).
                    PT0, sme0, smo0, ri0 = pre[0]
                    PT1, sme1, smo1, _ = pre[1]
                    PT2 = xp.tile([128, NT, 512], BF16, tag="xcb", name="PT2")
                    at0 = pat.tile([128, 512], F32, tag="at", name="at0")
                    at1 = ppj.tile([128, 512], F32, tag="pj", name="at1")
                    stot1 = stot_of(sme1, smo1, "1")
                    sme2 = smp.tile([128, 512], F32, tag="sme", name="sme2")
                    smo2 = smp.tile([128, 512], F32, tag="smo", name="smo2")
                    ri1 = None
                    for t in range(nsk):
                        score_exp(2, t, PT2, sme2, smo2, pre_sc(t))
                        pv(at0, 0, t, PT0)
                        if t == 3:
                            ri1 = rowsum(stot1, "1")
                    an0 = norm(at0, ri0, "0")
                    PT3 = ptp.tile([128, NT, 512], BF16, tag="pt", name="PT3")
                    stot2 = stot_of(sme2, smo2, "2")
                    sme3 = smp.tile([128, 512], F32, tag="sme", name="sme3")
                    smo3 = smp.tile([128, 512], F32, tag="smo", name="smo3")
                    ri2 = None
                    for t in range(nsk):
                        score_exp(3, t, PT3, sme3, smo3, pre_sc(t))
                        pv(at1, 1, t, PT1)
                        if t == 7:
                            ri2 = rowsum(stot2, "2")
                    an1 = norm(at1, ri1, "1")
                    at2 = ppj.tile([128, 512], F32, tag="pj", name="at2")
                    for t in range(nsk):
                        pv(at2, 2, t, PT2)
                    stot3 = stot_of(sme3, smo3, "3")
                    an2 = norm(at2, ri2, "2")
                    at3 = ppj.tile([128, 512], F32, tag="pj", name="at3")
                    for t in range(nsk):
                        pv(at3, 3, t, PT3)
                    ri3 = rowsum(stot3, "3")
                    an3 = norm(at3, ri3, "3")
                    attn_g = [an0, an1, an2, an3]
                else:
                    attn_g = []
                    for h in range(HL):
                        PTt = ptp.tile([128, NT, 512], BF16, tag="pt")
                        sme = smp.tile([128, 512], F32, tag="sme")
                        smo = smp.tile([128, 512], F32, tag="smo")
                        at = pat.tile([128, 512], F32, tag="at")
                        # per-partition partial sums of PT tiles (even/odd
                        # chains); the cross-partition reduction happens in
                        # ONE ones-matmul
                        for t in range(nsk):
                            sc = psc.tile([128, 512], F32, tag="sc")
                            score_exp(h, t, PTt, sme, smo, sc)
                            pv(at, h, t, PTt)
                        stot = stot_of(sme, smo, "")
                        ri = rowsum(stot, "")
                        an = norm(at, ri, "")
                        attn_g.append(an)

                # ---- wo partial for this q-block ----
                for j in range(4):
                    st = 4 * g + j
                    last_row = (g == NCH - 1 and j == 3)
                    for oc2 in range(4):
                        ysb = yp.tile([128, 1024], BF16, tag="ysb")
                        for half in range(2):
                            oc = 2 * oc2 + half
                            # 4-bank accumulator rotation: pyo's two banks
                            # plus pat and prs, both idle during wo. Deeper
                            # pipelining, and the first tiles don't wait for
                            # the attention tail to release pyo.
                            ypool, ytag = ((pat, "at") if (j * 8 + oc2 * 2 + half) % 4 == 0
                                           else (prs, "rs") if (j * 8 + oc2 * 2 + half) % 4 == 1
                                           else (pyo, "yo"))
                            yps = ypool.tile([128, 512], F32, tag=ytag, name="yps")
                            for h in range(HL):
                                nc.tensor.matmul(
                                    yps[:],
                                    attn_g[h][:, j * 128:(j + 1) * 128],
                                    woT[:, h, oc * 512:(oc + 1) * 512],
                                    start=(h == 0), stop=(h == HL - 1),
                                )
                            hs2 = slice(half * 512, (half + 1) * 512)
                            if half == 0:
                                nc.vector.tensor_copy(out=ysb[:, hs2], in_=yps[:])
                            else:
                                nc.scalar.copy(out=ysb[:, hs2], in_=yps[:])
                            if last_row:
                                # final row: store halves separately so the DMA
                                # of half 0 overlaps the copy of half 1
                                nc.sync.dma_start(
                                    out=y_t[st * 128:(st + 1) * 128, oc * 512:(oc + 1) * 512],
                                    in_=ysb[:, hs2],
                                )
                        if not last_row:
                            nc.sync.dma_start(
                                out=y_t[st * 128:(st + 1) * 128, oc2 * 1024:(oc2 + 1) * 1024],
                                in_=ysb[:],
                            )

    nc.compile()
    return nc


def _prep_inputs(x, freqs, wq, wk, wv, wo):
    x2 = np.asarray(x, dtype=np.float32).reshape(S, D)
    xT = np.ascontiguousarray(x2.T).astype(BFNP)
    f = np.asarray(freqs, dtype=np.float32)
    c64 = np.cos(f).T.astype(np.float32)   # [64, S]
    s64 = np.sin(f).T.astype(np.float32)
    cosT = np.ascontiguousarray(np.concatenate([c64, c64], axis=0))  # [128, S]
    sinT = np.ascontiguousarray(np.concatenate([s64, s64], axis=0))
    # pair permutation: evens then odds within each head's 128 rows
    perm = np.concatenate([np.arange(0, HD, 2), np.arange(1, HD, 2)])
    i = np.arange(128)
    maskT = (i[:, None] <= i[None, :]).astype(BFNP)  # keep sk <= sq
    in_maps = []
    for c in range(NCORES):
        sl = slice(c * DL, (c + 1) * DL)
        wq_c = wq[sl, :].reshape(HL, HD, D)[:, perm, :].reshape(DL, D)
        wk_c = wk[sl, :].reshape(HL, HD, D)[:, perm, :].reshape(DL, D)

        def pack_qk(w_c):
            # w_c [DL, D] -> transpose [D, DL] -> blocks [m, q4, r, kk, c]
            wT = np.ascontiguousarray(w_c.T)                  # [4096, 512]
            a = wT.reshape(KD // 4, 4, 128, HL, HD)            # [q4, kk, r, m, c]
            return np.ascontiguousarray(
                a.transpose(3, 0, 2, 1, 4).reshape(D, DL)).astype(BFNP)
        in_maps.append({
            "xT": xT,
            "cosT": cosT,
            "sinT": sinT,
            "wq": pack_qk(wq_c),
            "wk": pack_qk(wk_c),
            "wv": np.ascontiguousarray(wv[sl, :].T).astype(BFNP),
            "wo": np.ascontiguousarray(wo[:, sl].T).astype(BFNP),
            "maskT": maskT,
        })
    return in_maps


def _run(inputs, trace=False):
    if "nc" not in _CACHE:
        _CACHE["nc"] = _build()
    nc = _CACHE["nc"]
    in_maps = _prep_inputs(**inputs)
    res = run_bass_kernel_spmd(nc, in_maps, core_ids=list(range(NCORES)), trace=trace)
    y = np.zeros((S, D), dtype=np.float32)
    for c in range(NCORES):
        y += res.results[c]["y"].astype(np.float32)
    return y.reshape(B, S, D), res.exec_time_ns


def kernel(**inputs):
    y, _ = _run(inputs, trace=False)
    return y

